# revision 30
# baseline (speedup 1.0000x reference)
"""Trainium2 Bass kernel for causal self-attention (B=4, T=2048, C=1024, H=16).

Sharding: 8 cores = 4 batch-pairs x 2-way tensor parallel over heads.
Core c handles batch c//2 and heads [8*(c%2), 8*(c%2)+8).  Each core:
  phase 1: qkT = Wqk^T @ x^T (+bias)  [transposed-projection for Q,K]
           v   = x @ Wv' (+bias), Wv' has a ones column appended per head
                 (65 cols/head) so row-sums of P come free; V/P in bf16.
  phase 2: per head-pair, S^T = K^T' Q into a shared [128,1024] strip
           (both heads hh-major, so one ACT exp covers both), exp via ACT
           (scale=1/8, no max subtraction -- |S/8| < ~4), causal handled by
           N-restricted matmuls + an upper-tri mask multiply on DVE per
           diagonal subtile, O^T/L accumulated with lhsT=[V_h|1] (bf16).
  phase 3: norm: DVE evicts O^T to yT; 1/L = exp(-ln L) on ACT (one table
           set, no reloads), broadcast across partitions by a K=1 PE matmul
           (ones x recl), applied in-place on DVE.  Then
           out_partial = y @ Wp_local.  Host sums the two partials per
           batch and adds b_proj (the tensor-parallel all-reduce at gather).
Matmuls run as float32r (full-rate fp32 on the PE at free-dim >= 256);
P/V are bf16 (same PE rate, half the SBUF).  Loop bodies share one pool
scope so loop l+1's projection DMAs/matmuls pipeline under loop l's
attention tail (weights/x reload per loop; wp's DMA is deferred past the
previous loop's outproj reads).

HW notes driving this structure (measured on axon TRN2, where CoreSim's
cost model is ~accurate for engine throughput but not dispatch latency):
gpsimd ops cost 1.3-5us dispatch each (partition_broadcast/tensor_mul ->
moved to DVE/PE/DMA); DVE InstReciprocal ~3.3us per [1,512] op (-> ACT
ln/exp); act-table thrash costs ~1.3us per reload (-> single table set,
see build_kernel); DVE/ACT ops may read PSUM only via one operand and
partition bases must be 32-aligned, but shifted in/out bases are fine.
"""

import os
import sys

sys.path.insert(0, "/opt/trn_rl_repo")

import numpy as np

import concourse.bass as bass
import concourse.tile as tile
from concourse import bacc, mybir
from concourse.bass_utils import run_bass_kernel_spmd

F32 = mybir.dt.float32
F32R = mybir.dt.float32r
BF16 = mybir.dt.bfloat16
AF = mybir.ActivationFunctionType

B, T, C, H, HD = 4, 2048, 1024, 16, 64
N_CORES = 8
HL = 8          # heads per core
VW = HL * (HD + 1)  # 520: v strip width (64 cols + ones col per head)

LAST_RESULT = None  # BassKernelResults of the most recent run (for test.py)
_CACHED = None      # (nc,) build cache


def build_kernel(loops=1):
    # The act-table chooser first-matches each activation against
    # act_info.json set order, so exp lands in set 0 (exp_and_others) while
    # ln needs set 1 (natural_log_exp_and_others) -> ~52 table reloads per
    # loop.  Emptying set 0 for the duration of the build steers every
    # activation (exp/copy/ln) to set 1, which serves them all: one load
    # total.  Indices are preserved, so the emitted act_func_set_id still
    # matches act_info.json.
    import concourse.hw_specs as _hs
    import concourse.bacc as _bacc_mod

    _orig_tables = _bacc_mod.get_activation_tables

    def _tables_no_set0(arch):
        items = list(_orig_tables(arch).items())
        items[0] = (items[0][0], set())
        return dict(items)

    _bacc_mod.get_activation_tables = _tables_no_set0
    try:
        return _build_kernel_inner(loops)
    finally:
        _bacc_mod.get_activation_tables = _orig_tables


def _build_kernel_inner(loops=1):
    nc = bacc.Bacc(
        "TRN2",
        target_bir_lowering=False,
        debug=False,
        enable_asserts=False,
        num_devices=N_CORES,
    )
    d_xT = nc.dram_tensor("xT", [C, T], F32, kind="ExternalInput").ap()
    d_wqk = nc.dram_tensor("wqk", [C, C], F32, kind="ExternalInput").ap()
    d_wv = nc.dram_tensor("wv", [C, VW], F32, kind="ExternalInput").ap()
    d_bqk = nc.dram_tensor("bqk", [C], F32, kind="ExternalInput").ap()
    d_bv = nc.dram_tensor("bv", [VW], F32, kind="ExternalInput").ap()
    d_wp = nc.dram_tensor("wp", [HL * HD, C], BF16, kind="ExternalInput").ap()
    d_mask = nc.dram_tensor("mask", [128, 128], BF16, kind="ExternalInput").ap()
    d_ones = nc.dram_tensor("ones", [128], F32, kind="ExternalInput").ap()
    d_out = nc.dram_tensor("out", [T, C], F32, kind="ExternalOutput").ap()

    with tile.TileContext(nc) as tc:
        kernel_body(tc, d_xT, d_wqk, d_wv, d_bqk, d_bv, d_wp, d_mask, d_ones, d_out, loops)
    nc.compile()
    return nc


def kernel_body(tc, d_xT, d_wqk, d_wv, d_bqk, d_bv, d_wp, d_mask, d_ones, d_out, loops):
    nc = tc.nc
    from contextlib import ExitStack

    ctx = ExitStack()
    with ctx:
        # ---- pools (stack allocator: persistent first) ----
        p_misc = ctx.enter_context(tc.tile_pool(name="misc", bufs=1))
        p_kT = ctx.enter_context(tc.tile_pool(name="kT", bufs=1))
        p_v = ctx.enter_context(tc.tile_pool(name="v", bufs=1))
        p_w1 = ctx.enter_context(tc.tile_pool(name="w1", bufs=1))
        p_q = ctx.enter_context(tc.tile_pool(name="q", bufs=2))
        p_yT = ctx.enter_context(tc.tile_pool(name="yT", bufs=2))
        p_xt = ctx.enter_context(tc.tile_pool(name="xt", bufs=2))
        p_pt = ctx.enter_context(tc.tile_pool(name="pt", bufs=2))
        p_sm = ctx.enter_context(tc.tile_pool(name="sm", bufs=1))
        p_po = ctx.enter_context(tc.tile_pool(name="po", bufs=2))
        p_ps = ctx.enter_context(tc.tile_pool(name="ps", bufs=2, space="PSUM"))
        p_ps_s = ctx.enter_context(tc.tile_pool(name="ps_s", bufs=2, space="PSUM"))
        p_ps_o = ctx.enter_context(tc.tile_pool(name="ps_o", bufs=1, space="PSUM"))

        mask_s = p_misc.tile([128, 128], BF16)
        nc.sync.dma_start(out=mask_s, in_=d_mask)
        ones_s = p_misc.tile([1, 128], F32R)
        nc.sync.dma_start(
            out=ones_s, in_=d_ones.rearrange("(o a) -> o a", o=1).bitcast(F32R)
        )
        bqk_s = p_misc.tile([128, 8], F32)
        nc.sync.dma_start(out=bqk_s, in_=d_bqk.rearrange("(a p) -> p a", p=128))
        bv_s = p_misc.tile([1, VW], F32R)
        nc.sync.dma_start(out=bv_s, in_=d_bv.rearrange("(o a) -> o a", o=1).bitcast(F32R))
        bvb_s = p_misc.tile([128, VW], F32R)
        # broadcast bias across partitions with a stride-0 DMA (gpsimd ops
        # have multi-us dispatch latency on this hw)
        nc.sync.dma_start(
            out=bvb_s,
            in_=d_bv.rearrange("(o a) -> o a", o=1).bitcast(F32R).partition_broadcast(128),
        )

        q_tiles = {}
        yT_tiles = {}
        cur = {}   # per-loop persistent tiles: kT, v, wqk, wv, wp

        def proj_items(s):
            tt = s % 4
            xt_s = p_xt.tile([128, 8, 512], F32R, tag="xt", name="xt_s")
            if tt == 0:
                # fresh per-loop tiles; tag rotation (bufs=1) serializes
                # against the previous loop's last readers
                cur["kT"] = p_kT.tile([128, 4, T], F32R, tag="kT", name="kT_s")
                cur["v"] = p_v.tile([128, 16, VW], BF16, tag="v", name="v_s")
                cur["wqk"] = p_w1.tile([128, 8, C], F32R, tag="wqk", name="wqk_s")
                cur["wv"] = p_w1.tile([128, 8, VW], F32R, tag="wv", name="wv_s")
                cur["wp"] = p_w1.tile([128, 4, C], BF16, tag="wp", name="wp_s")
                # wp DMA is deferred to the end of this slot: outproj of the
                # previous loop still reads the old wp buffer while this
                # slot's items are being emitted (tag rotation would race)
                def wp_dma(wp_t=cur["wp"]):
                    for i in range(4):
                        nc.sync.dma_start(
                            out=wp_t[:, i, :],
                            in_=d_wp[128 * i:128 * i + 128, :],
                        )
                cur["wp_dma"] = wp_dma
                for i in range(8):
                    nc.sync.dma_start(
                        out=cur["wqk"][:, i, :],
                        in_=d_wqk[128 * i:128 * i + 128, :].bitcast(F32R),
                    )
                    nc.sync.dma_start(
                        out=xt_s[:, i, :],
                        in_=d_xT[128 * i:128 * i + 128, 0:512].bitcast(F32R),
                    )
                for i in range(8):
                    nc.sync.dma_start(
                        out=cur["wv"][:, i, :],
                        in_=d_wv[128 * i:128 * i + 128, :].bitcast(F32R),
                    )
            else:
                for i in range(8):
                    nc.sync.dma_start(
                        out=xt_s[:, i, :],
                        in_=d_xT[128 * i:128 * i + 128, 512 * tt:512 * tt + 512].bitcast(F32R),
                    )
            kT_s, v_s, wqk_s, wv_s = cur["kT"], cur["v"], cur["wqk"], cur["wv"]
            q_s = p_q.tile([128, 4, 512], F32R, tag="q", name="q_s")
            q_tiles[s] = q_s
            items = []

            def qk_group(j):
                ps = p_ps.tile([128, 512], F32, tag="p1", name="ps_p1")
                for i in range(8):
                    nc.tensor.matmul(
                        ps,
                        lhsT=wqk_s[:, i, 128 * j:128 * j + 128],
                        rhs=xt_s[:, i, :],
                        start=(i == 0),
                        stop=(i == 7),
                    )
                dest = (
                    q_s[:, j, :] if j < 4
                    else kT_s[:, j - 4, 512 * tt:512 * tt + 512]
                )
                nc.vector.tensor_scalar_add(out=dest, in0=ps, scalar1=bqk_s[:, j:j + 1])

            def v_group(st, half):
                ts_ = 4 * tt + st
                psv = p_ps.tile([128, 260], F32, tag="p1", name="ps_v")
                for i in range(8):
                    nc.tensor.matmul(
                        psv,
                        lhsT=xt_s[:, i, 128 * st:128 * st + 128],
                        rhs=wv_s[:, i, 260 * half:260 * half + 260],
                        start=(i == 0),
                        stop=(i == 7),
                    )
                nc.vector.tensor_add(
                    out=v_s[:, ts_, 260 * half:260 * half + 260],
                    in0=psv,
                    in1=bvb_s[:, 260 * half:260 * half + 260],
                )

            qk_items = []
            if tt == 0:
                # Q groups first at loop starts: their evictions (q, bufs=2)
                # don't wait on the previous loop, while kT evictions must
                # wait for the previous loop's last S^T reads
                for j in range(4):
                    qk_items.append(lambda j=j: qk_group(j))
                for j in range(4, 8):
                    qk_items.append(lambda j=j: qk_group(j))
            else:
                # K chunk then Q chunk per pair: unblocks attention earliest
                for p4 in range(4):
                    qk_items.append(lambda j=4 + p4: qk_group(j))
                    qk_items.append(lambda j=p4: qk_group(j))
            v_items = []
            for st in range(4):
                for half in range(2):
                    v_items.append(lambda st=st, half=half: v_group(st, half))
            return qk_items, v_items

        def attn_items(s):
            """Returns list of (callable, req_qk, req_v): req_qk/req_v are how many
            of this slot's qk/v groups must be emitted before this item."""
            qt = s % 4
            q_s = q_tiles[s]
            kT_s, v_s = cur["kT"], cur["v"]
            yT_b = p_yT.tile([128, 4, 512], BF16, tag="yT", name="yT_b")
            yT_tiles[s] = yT_b
            items = []
            o_tiles = {}

            def attn_group(p4, kr):
                if kr == 0:
                    # one [65,1024] tile for both heads (hh-major halves), so
                    # the norm's ln/exp each run once over both L rows
                    o_tiles[p4] = p_ps_o.tile([65, 1024], F32, tag="o", name="o")
                o_ps = o_tiles[p4]
                p = kr - 4 * qt
                n0 = 128 * p if p > 0 else 0
                # both heads share one [128,1024] strip (hh-major) so each
                # exp covers both heads in one ACT instruction; bufs=2 keeps
                # the next chunk's S^T from waiting on this chunk's exp
                strip = p_ps_s.tile([128, 1024], F32, tag="s", name="strip")
                for hh in range(2):
                    poff = 64 * hh
                    nc.tensor.matmul(
                        strip[:, 512 * hh + n0:512 * hh + 512],
                        lhsT=kT_s[poff:poff + 64, p4, 128 * kr:128 * kr + 128],
                        rhs=q_s[poff:poff + 64, p4, n0:512],
                        start=True,
                        stop=True,
                        tile_position=(poff, 0),
                    )
                pt_t = p_pt.tile([128, 1024], BF16, tag="pt", name="pt")
                strip_v = strip.rearrange("p (h c) -> p h c", h=2)
                pt_v = pt_t.rearrange("p (h c) -> p h c", h=2)
                if p >= 0:
                    nc.scalar.activation(
                        out=pt_v[:, :, n0:512],
                        in_=strip_v[:, :, n0:512],
                        func=AF.Exp,
                        scale=0.125,
                    )
                    nc.vector.tensor_mul(
                        out=pt_v[:, :, n0:n0 + 128],
                        in0=pt_v[:, :, n0:n0 + 128],
                        in1=mask_s.rearrange("p (o c) -> p o c", o=1).broadcast_to([128, 2, 128]),
                    )
                else:
                    nc.scalar.activation(
                        out=pt_t, in_=strip, func=AF.Exp, scale=0.125
                    )
                for hh in range(2):
                    hl = 2 * p4 + hh
                    nc.tensor.matmul(
                        o_ps[0:65, 512 * hh + n0:512 * hh + 512],
                        lhsT=v_s[:, kr, 65 * hl:65 * hl + 65],
                        rhs=pt_t[:, 512 * hh + n0:512 * hh + 512],
                        start=(kr == 0),
                        stop=(kr == 4 * qt + 3),
                    )

            def norm(p4):
                # 1/L broadcast via a K=1 PE matmul (ones ⊗ recl) into a dead
                # strip bank; gpsimd broadcast + sbuf-dma cost multi-us each
                # on hw.  hh=1's mul writes partitions 64-127 directly (DVE
                # supports shifted partition bases).
                o_ps = o_tiles[p4]
                recl = p_sm.tile([1, 1024], F32R, tag="recl", name="recl")
                for hh in range(2):
                    # DVE evicts unnormalized O^T straight into yT (shifted
                    # partition bases are fine); scale joins in-place below,
                    # since DVE can't read two PSUM operands.
                    nc.vector.tensor_scalar_add(
                        out=yT_b[64 * hh:64 * hh + 64, p4, :],
                        in0=o_ps[0:64, 512 * hh:512 * hh + 512],
                        scalar1=0.0,
                    )
                # 1/L = exp(-ln L) on ACT over BOTH heads' L rows at once:
                # both funcs live in the natural_log_exp table set (no table
                # reload), unlike DVE reciprocal (~3.3us per op on this hw).
                with nc.allow_low_precision(
                    reason="1/L feeds an f32r broadcast matmul"
                ):
                    nc.scalar.activation(
                        out=recl, in_=o_ps[64:65, :], func=AF.Ln, scale=1.0,
                    )
                    nc.scalar.activation(
                        out=recl, in_=recl, func=AF.Exp, scale=-1.0,
                    )
                # 1/L broadcast via K=1 PE matmuls (ones ⊗ recl) into the
                # o-tag psum rotation (couples to the o_ps lifetime, not the
                # S^T strip rotation); matmul dst partition base must be 0,
                # and each matmul output must stay within one psum bank.
                lbt = p_ps_o.tile([65, 1024], F32, tag="o", name="lb")
                for hh in range(2):
                    nc.tensor.matmul(
                        lbt[0:64, 512 * hh:512 * hh + 512],
                        lhsT=ones_s[0:1, 0:64],
                        rhs=recl[0:1, 512 * hh:512 * hh + 512],
                        start=True,
                        stop=True,
                    )
                    nc.vector.tensor_mul(
                        out=yT_b[64 * hh:64 * hh + 64, p4, :],
                        in0=yT_b[64 * hh:64 * hh + 64, p4, :],
                        in1=lbt[0:64, 512 * hh:512 * hh + 512],
                    )

            for p4 in range(4):
                rq = 2 * p4 + 2      # qk groups up to and incl this pair's K,Q
                for kr in range(4 * qt + 4):
                    diag = kr >= 4 * qt
                    items.append(
                        (lambda p4=p4, kr=kr: attn_group(p4, kr), rq, 8 if diag else 0)
                    )
                items.append((lambda p4=p4: norm(p4), rq, 8))
            return items

        def outproj_items(s, wp_s):
            qt = s % 4
            yT_b = yT_tiles[s]
            items = []

            def out_group(st, half):
                ts_ = 4 * qt + st
                ps = p_ps.tile([128, 512], F32, tag="p1", name="ps_out")
                for cc in range(4):
                    nc.tensor.matmul(
                        ps,
                        lhsT=yT_b[:, cc, 128 * st:128 * st + 128],
                        rhs=wp_s[:, cc, 512 * half:512 * half + 512],
                        start=(cc == 0),
                        stop=(cc == 3),
                    )
                ot = p_po.tile([128, 512], F32, tag="ot", name="ot")
                # DVE eviction: ACT is the scarcer engine now (exp + 1/L)
                nc.vector.tensor_scalar_add(out=ot, in0=ps, scalar1=0.0)
                nc.sync.dma_start(
                    out=d_out[128 * ts_:128 * ts_ + 128, 512 * half:512 * half + 512],
                    in_=ot,
                )

            for st in range(4):
                for half in range(2):
                    items.append(lambda st=st, half=half: out_group(st, half))
            return items

        # Flat cross-loop pipeline: proj(s) groups feed attention(s) with
        # dep-aware merge; outproj(s-1) groups are sprinkled through slot s,
        # including across loop boundaries (outproj of (l-1, qt=3) runs
        # under (l, qt=0)'s proj/attention).
        prev_wp = None
        for s in range(4 * loops):
            qt = s % 4
            qk_items, v_items = proj_items(s)
            b_items = attn_items(s)
            o_items = outproj_items(s - 1, prev_wp) if s >= 1 else []
            prev_wp = cur["wp"]
            ia = iv = io = 0
            if qt == 0:
                # all of slot 0's attention is diagonal (needs V): run the qk
                # matmuls first so the PE isn't stalled on the wv DMAs
                while ia < len(qk_items):
                    qk_items[ia](); ia += 1
            nb = len(b_items)
            for k, (fn, rq, rv) in enumerate(b_items):
                while ia < rq:
                    qk_items[ia](); ia += 1
                while iv < rv:
                    v_items[iv](); iv += 1
                # sprinkle leftovers proportionally to attention progress
                while io < len(o_items) * (k + 1) // nb:
                    o_items[io](); io += 1
                target_a = min(len(qk_items), 2 + (len(qk_items) - 2) * (k + 1) // nb)
                while ia < target_a:
                    qk_items[ia](); ia += 1
                target_v = min(len(v_items), 8 * (k + 1) // max(1, nb - 4))
                while iv < target_v:
                    v_items[iv](); iv += 1
                fn()
            while ia < len(qk_items):
                qk_items[ia](); ia += 1
            while iv < len(v_items):
                v_items[iv](); iv += 1
            while io < len(o_items):
                o_items[io](); io += 1
            if qt == 0:
                cur["wp_dma"]()
        for f in outproj_items(4 * loops - 1, prev_wp):
            f()


def make_core_inputs(x, W_attn, b_attn, W_proj):
    f = np.float32
    mask = np.triu(np.ones((128, 128), f)).astype(mybir.dt.np(BF16))
    in_maps = []
    for c in range(N_CORES):
        b, g = divmod(c, 2)
        hs = range(HL * g, HL * g + HL)
        xT = np.ascontiguousarray(x[b].T)
        wq = np.concatenate([W_attn[:, h * HD:h * HD + HD] for h in hs], axis=1)
        wk = np.concatenate([W_attn[:, C + h * HD:C + h * HD + HD] for h in hs], axis=1)
        wqk = np.ascontiguousarray(np.concatenate([wq, wk], axis=1))
        bq = np.concatenate([b_attn[h * HD:h * HD + HD] for h in hs])
        bk = np.concatenate([b_attn[C + h * HD:C + h * HD + HD] for h in hs])
        bqk = np.ascontiguousarray(np.concatenate([bq, bk]))
        wv = np.zeros((C, VW), f)
        bv = np.zeros((VW,), f)
        for i, h in enumerate(hs):
            wv[:, 65 * i:65 * i + 64] = W_attn[:, 2 * C + h * HD:2 * C + h * HD + HD]
            bv[65 * i:65 * i + 64] = b_attn[2 * C + h * HD:2 * C + h * HD + HD]
            bv[65 * i + 64] = 1.0
        wp = np.ascontiguousarray(
            np.concatenate([W_proj[h * HD:h * HD + HD, :] for h in hs], axis=0)
        ).astype(mybir.dt.np(BF16))
        in_maps.append(
            {"xT": xT, "wqk": wqk, "wv": wv, "bqk": bqk, "bv": bv, "wp": wp,
             "mask": mask, "ones": np.ones((128,), f)}
        )
    return in_maps


def kernel(**inputs):
    global LAST_RESULT, _CACHED
    f = np.float32
    x = np.asarray(inputs["x"], f)
    W_attn = np.asarray(inputs["W_attn"], f)
    b_attn = np.asarray(inputs["b_attn"], f)
    W_proj = np.asarray(inputs["W_proj"], f)
    b_proj = np.asarray(inputs["b_proj"], f)

    if _CACHED is None:
        _CACHED = build_kernel()
    nc = _CACHED
    in_maps = make_core_inputs(x, W_attn, b_attn, W_proj)
    res = run_bass_kernel_spmd(nc, in_maps, core_ids=list(range(N_CORES)))
    LAST_RESULT = res
    y = np.empty((B, T, C), f)
    for b in range(B):
        y[b] = res.results[2 * b]["out"] + res.results[2 * b + 1]["out"] + b_proj
    return y

